# revision 5
# baseline (speedup 1.0000x reference)
"""Trainium2 Bass kernel for ContrastCELoss (weighted CE + SupCon pixel contrast).

Strategy (8 NeuronCores, SPMD):
  L1 (core b = batch b): weighted-CE partial sums; global top-512 fg/bg
      thresholds for cam via branchless binary search; fg/bg masks.
  host: mask -> index lists (pure index bookkeeping).
  L2 (core b = batch b): PE-transpose embed [256,16384] -> DRAM pixel-major
      [16384,256]; indirect-DMA gather of the 1024 selected rows;
      PE-transpose back to channel-major feature block [256,1024]; ||f||^2.
  host: concat feature blocks -> all_featsT [256,8192] (fg rows then bg rows).
  L3 (core c = anchor-row block c): Gram block [1024,8192] on the PE; row
      stats (max, exp-sums per half, log-term sums per half) for the SupCon
      loss, row-parallel (SupCon row-parallel sharding).
  host: exact fp32 finalization reproducing jax semantics, including the
      neg_sum==0 underflow -> 0*inf -> NaN propagation of the reference.
"""
import os
import time

import numpy as np

import concourse.bass as bass
import concourse.mybir as mybir
from concourse import bacc
from concourse.tile import TileContext
from concourse.masks import make_identity

fp32 = mybir.dt.float32
bf16 = mybir.dt.bfloat16
i32 = mybir.dt.int32
u8 = mybir.dt.uint8
AF = mybir.ActivationFunctionType
ALU = mybir.AluOpType
AX = mybir.AxisListType

TEMP = 0.07
BASE_TEMP = 0.07
LOSS_WEIGHT = 0.1
NUM_SAMPLES = 512
IGNORE_INDEX = 255

B, C, H, W = 8, 21, 128, 128
NPIX = H * W          # 16384
D = 256               # embed dim
N = 2 * B * NUM_SAMPLES  # 8192 contrast rows
HALF = N // 2         # 4096 (fg half / bg half)
NROW = N // 8         # 1024 anchor rows per core
SEARCH_ITERS = 36

import ml_dtypes

FEAT_DT = bf16 if os.environ.get("KERNEL_FEAT_BF16", "0") == "1" else fp32
FEAT_NP = ml_dtypes.bfloat16 if FEAT_DT is bf16 else np.float32

_cache: dict = {}
last_exec_ns: dict = {}


def _new_nc():
    return bacc.Bacc("TRN2", target_bir_lowering=False, debug=False,
                     num_devices=8)


# --------------------------------------------------------------------------
# L1: CE partials + cam thresholds + masks (per-batch data parallel)
# --------------------------------------------------------------------------
def _build_l1():
    nc = _new_nc()
    cam_d = nc.dram_tensor("cam", [H, W], fp32, kind="ExternalInput")
    seg_d = nc.dram_tensor("seg", [C, H, W], fp32, kind="ExternalInput")
    tgt_d = nc.dram_tensor("tgt", [H, W], i32, kind="ExternalInput")
    cw_d = nc.dram_tensor("cw", [C], fp32, kind="ExternalInput")
    thr_o = nc.dram_tensor("thr", [H, 2], fp32, kind="ExternalOutput")
    cnt_o = nc.dram_tensor("cnt", [H, 2], fp32, kind="ExternalOutput")
    fg_o = nc.dram_tensor("fgm", [H, W], u8, kind="ExternalOutput")
    bg_o = nc.dram_tensor("bgm", [H, W], u8, kind="ExternalOutput")
    ce_o = nc.dram_tensor("ce", [H, 2], fp32, kind="ExternalOutput")

    with TileContext(nc) as tc:
        with (
            tc.tile_pool(name="per", bufs=1) as per,
            tc.tile_pool(name="mid", bufs=2) as midp,
            tc.tile_pool(name="ge", bufs=2) as gep,
            tc.tile_pool(name="pr", bufs=2) as prp,
            tc.tile_pool(name="ps", bufs=2, space="PSUM") as psp,
            tc.tile_pool(name="ce", bufs=2) as cep,
        ):
            camt = per.tile([H, W], fp32, tag="camt")
            nc.sync.dma_start(out=camt[:], in_=cam_d[:])
            ncamt = per.tile([H, W], fp32, tag="ncamt")
            nc.vector.tensor_scalar_mul(ncamt[:], camt[:], -1.0)
            ones_t = per.tile([128, 128], fp32, tag="ones")
            nc.vector.memset(ones_t[:], 1.0)

            # ping-pong [128,2] interval tiles (col 0: fg on cam, col 1: bg on -cam)
            lo = [per.tile([128, 2], fp32, tag=f"lo{i}", name=f"lo{i}") for i in range(2)]
            hi = [per.tile([128, 2], fp32, tag=f"hi{i}", name=f"hi{i}") for i in range(2)]
            nc.vector.memset(lo[0][:], -20.0)
            nc.vector.memset(hi[0][:], 20.0)
            cnt2 = per.tile([128, 2], fp32, tag="cnt2")

            for it in range(SEARCH_ITERS):
                cur, nxt = lo[it % 2], lo[(it + 1) % 2]
                curh, nxth = hi[it % 2], hi[(it + 1) % 2]
                mid = midp.tile([128, 2], fp32, tag="mid")
                nc.vector.tensor_tensor(out=mid[:], in0=cur[:], in1=curh[:],
                                        op=ALU.add)
                nc.vector.tensor_scalar_mul(mid[:], mid[:], 0.5)
                gef = gep.tile([128, W], fp32, tag="gef")
                nc.vector.tensor_scalar(gef[:], camt[:], mid[:, 0:1], None,
                                        op0=ALU.is_ge, op1=ALU.add,
                                        accum_out=cnt2[:, 0:1])
                geb = gep.tile([128, W], fp32, tag="geb")
                nc.vector.tensor_scalar(geb[:], ncamt[:], mid[:, 1:2], None,
                                        op0=ALU.is_ge, op1=ALU.add,
                                        accum_out=cnt2[:, 1:2])
                csum = psp.tile([128, 2], fp32)
                nc.tensor.matmul(csum[:], ones_t[:], cnt2[:], start=True,
                                 stop=True)
                pred = prp.tile([128, 2], u8, tag="pred")
                nc.vector.tensor_scalar(pred[:], csum[:], float(NUM_SAMPLES),
                                        None, op0=ALU.is_ge)
                nc.vector.select(nxt[:], pred[:], mid[:], cur[:])
                nc.vector.select(nxth[:], pred[:], curh[:], mid[:])

            flo = lo[SEARCH_ITERS % 2]
            nc.sync.dma_start(out=thr_o[:], in_=flo[:])
            # masks + exact counts
            cntf = per.tile([128, 2], fp32, tag="cntf")
            fgf = per.tile([H, W], fp32, tag="fgf")
            nc.vector.tensor_scalar(fgf[:], camt[:], flo[:, 0:1], None,
                                    op0=ALU.is_ge, op1=ALU.add,
                                    accum_out=cntf[:, 0:1])
            bgf = per.tile([H, W], fp32, tag="bgf")
            nc.vector.tensor_scalar(bgf[:], ncamt[:], flo[:, 1:2], None,
                                    op0=ALU.is_ge, op1=ALU.add,
                                    accum_out=cntf[:, 1:2])
            fgu = per.tile([H, W], u8, tag="fgu")
            nc.vector.tensor_copy(fgu[:], fgf[:])
            nc.sync.dma_start(out=fg_o[:], in_=fgu[:])
            bgu = per.tile([H, W], u8, tag="bgu")
            nc.vector.tensor_copy(bgu[:], bgf[:])
            nc.sync.dma_start(out=bg_o[:], in_=bgu[:])
            ctot = psp.tile([128, 2], fp32)
            nc.tensor.matmul(ctot[:], ones_t[:], cntf[:], start=True, stop=True)
            ctos = per.tile([128, 2], fp32, tag="ctos")
            nc.vector.tensor_copy(ctos[:], ctot[:])
            nc.sync.dma_start(out=cnt_o[:], in_=ctos[:])

            # ---- weighted CE ----
            T = cep.tile([H, C, W], fp32, tag="T")
            nc.sync.dma_start(out=T[:], in_=seg_d[:].rearrange("c h w -> h c w"))
            tg = cep.tile([H, W], i32, tag="tg")
            nc.sync.dma_start(out=tg[:], in_=tgt_d[:])
            tgf = cep.tile([H, W], fp32, tag="tgf")
            nc.vector.tensor_copy(tgf[:], tg[:])
            e3 = cep.tile([H, C, W], fp32, tag="e3")
            nc.scalar.activation(e3[:], T[:], AF.Exp)
            S = cep.tile([H, W], fp32, tag="S")
            nc.vector.tensor_reduce(S[:], e3[:].rearrange("p c w -> p w c"),
                                    axis=AX.X, op=ALU.add)
            logS = cep.tile([H, W], fp32, tag="logS")
            nc.scalar.activation(logS[:], S[:], AF.Ln)
            ioc = cep.tile([H, C, W], i32, tag="ioc")
            nc.gpsimd.iota(ioc[:], pattern=[[1, C], [0, W]], base=0,
                           channel_multiplier=0)
            iocf = cep.tile([H, C, W], fp32, tag="iocf")
            nc.vector.tensor_copy(iocf[:], ioc[:])
            tgb = tgf[:].rearrange("p (o w) -> p o w", o=1).to_broadcast(
                [H, C, W])
            match = cep.tile([H, C, W], fp32, tag="match")
            nc.vector.tensor_tensor(out=match[:], in0=iocf[:], in1=tgb,
                                    op=ALU.is_equal)
            prod = cep.tile([H, C, W], fp32, tag="prod")
            nc.vector.tensor_tensor(out=prod[:], in0=T[:], in1=match[:],
                                    op=ALU.mult)
            sat = cep.tile([H, W], fp32, tag="sat")
            nc.vector.tensor_reduce(sat[:], prod[:].rearrange("p c w -> p w c"),
                                    axis=AX.X, op=ALU.add)
            cwt = cep.tile([1, C], fp32, tag="cwt")
            nc.sync.dma_start(out=cwt[:1, :], in_=cw_d[None, :])
            cwb = cep.tile([128, C], fp32, tag="cwb")
            nc.gpsimd.partition_broadcast(cwb[:], cwt[:1, :])
            cwbb = cwb[:].rearrange("p (c o) -> p c o", o=1).to_broadcast(
                [H, C, W])
            wprod = cep.tile([H, C, W], fp32, tag="wprod")
            nc.vector.tensor_tensor(out=wprod[:], in0=match[:], in1=cwbb,
                                    op=ALU.mult)
            wpix = cep.tile([H, W], fp32, tag="wpix")
            nc.vector.tensor_reduce(wpix[:], wprod[:].rearrange("p c w -> p w c"),
                                    axis=AX.X, op=ALU.add)
            valid = cep.tile([H, W], fp32, tag="valid")
            nc.vector.tensor_scalar(valid[:], tgf[:], float(IGNORE_INDEX),
                                    None, op0=ALU.not_equal)
            nll = cep.tile([H, W], fp32, tag="nll")
            nc.vector.tensor_tensor(out=nll[:], in0=logS[:], in1=sat[:],
                                    op=ALU.subtract)
            nc.vector.tensor_tensor(out=nll[:], in0=nll[:], in1=valid[:],
                                    op=ALU.mult)
            nc.vector.tensor_tensor(out=wpix[:], in0=wpix[:], in1=valid[:],
                                    op=ALU.mult)
            wnll = cep.tile([H, W], fp32, tag="wnll")
            nc.vector.tensor_tensor(out=wnll[:], in0=nll[:], in1=wpix[:],
                                    op=ALU.mult)
            ce2 = cep.tile([H, 2], fp32, tag="ce2")
            nc.vector.tensor_reduce(ce2[:, 0:1], wnll[:], axis=AX.X, op=ALU.add)
            nc.vector.tensor_reduce(ce2[:, 1:2], wpix[:], axis=AX.X, op=ALU.add)
            nc.sync.dma_start(out=ce_o[:], in_=ce2[:])
    nc.compile()
    return nc


# --------------------------------------------------------------------------
# L2: embed transpose -> pixel-major DRAM -> gather -> featsT block
# --------------------------------------------------------------------------
def _build_l2():
    nc = _new_nc()
    emb_d = nc.dram_tensor("embed", [D, NPIX], fp32, kind="ExternalInput")
    idx_d = nc.dram_tensor("idx", [8, 128], i32, kind="ExternalInput")
    ft_o = nc.dram_tensor("ft", [2, 128, 1024], FEAT_DT, kind="ExternalOutput")
    nrm_o = nc.dram_tensor("nrm", [128, 8], fp32, kind="ExternalOutput")
    fpm = nc.dram_tensor("fpm", [NPIX, D], FEAT_DT)  # internal scratch

    with TileContext(nc) as tc:
        with (
            tc.tile_pool(name="per", bufs=1) as per,
            tc.tile_pool(name="emb", bufs=2) as embp,
            tc.tile_pool(name="tp", bufs=4, space="PSUM") as tpp,
            tc.tile_pool(name="st", bufs=8) as stp,
            tc.tile_pool(name="ps2", bufs=2, space="PSUM") as ps2,
        ):
            ident = per.tile([128, 128], fp32, tag="ident")
            make_identity(nc, ident[:])
            if FEAT_DT is not fp32:
                identf = per.tile([128, 128], FEAT_DT, tag="identf")
                make_identity(nc, identf[:])
            else:
                identf = ident
            idxt = per.tile([128, 8], i32, tag="idxt")
            nc.sync.dma_start(out=idxt[:], in_=idx_d[:].rearrange("g p -> p g"))

            fpm_v = fpm[:].rearrange("(a p) d -> p a d", p=128)  # [128,128,256]
            for cc in range(2):
                et = embp.tile([128, NPIX], fp32, tag="et")
                for s in range(4):
                    nc.sync.dma_start(
                        out=et[:, s * 4096:(s + 1) * 4096],
                        in_=emb_d[cc * 128:(cc + 1) * 128,
                                  s * 4096:(s + 1) * 4096])
                for pg in range(32):
                    pt = tpp.tile([128, 512], fp32, tag="pt")
                    for i in range(4):
                        pb = pg * 4 + i
                        nc.tensor.transpose(pt[:, i * 128:(i + 1) * 128],
                                            et[:, pb * 128:(pb + 1) * 128],
                                            ident[:])
                    ot = stp.tile([128, 4, 128], FEAT_DT, tag="ot")
                    nc.vector.tensor_copy(
                        ot[:], pt[:].rearrange("p (a d) -> p a d", a=4))
                    nc.sync.dma_start(
                        out=fpm_v[:, pg * 4:(pg + 1) * 4,
                                  cc * 128:(cc + 1) * 128],
                        in_=ot[:])

            # gather the 1024 selected pixel rows + transpose back
            gat = per.tile([128, 8, D], FEAT_DT, tag="gat")
            nrmt = per.tile([128, 8], fp32, tag="nrmt")
            ftt = [per.tile([128, 1024], FEAT_DT, tag=f"ftt{c}", name=f"ftt{c}")
                   for c in range(2)]
            for g in range(8):
                nc.gpsimd.indirect_dma_start(
                    out=gat[:, g, :], out_offset=None, in_=fpm[:],
                    in_offset=bass.IndirectOffsetOnAxis(ap=idxt[:, g:g + 1],
                                                        axis=0))
                sq = stp.tile([128, D], fp32, tag="sq")
                nc.scalar.activation(sq[:], gat[:, g, :], AF.Square,
                                     accum_out=nrmt[:, g:g + 1])
                for cc2 in range(2):
                    p2 = ps2.tile([128, 128], fp32, tag="p2")
                    nc.tensor.transpose(p2[:],
                                        gat[:, g, cc2 * 128:(cc2 + 1) * 128],
                                        identf[:])
                    nc.vector.tensor_copy(ftt[cc2][:, g * 128:(g + 1) * 128],
                                          p2[:])
            for cc2 in range(2):
                nc.sync.dma_start(out=ft_o[cc2], in_=ftt[cc2][:])
            nc.sync.dma_start(out=nrm_o[:], in_=nrmt[:])
    nc.compile()
    return nc


# --------------------------------------------------------------------------
# L3: row-block contrastive stats
# --------------------------------------------------------------------------
def _build_l3():
    nc = _new_nc()
    fa_d = nc.dram_tensor("featsT", [2, 128, N], FEAT_DT, kind="ExternalInput")
    rt_d = nc.dram_tensor("rowsT", [2, 128, NROW], FEAT_DT,
                          kind="ExternalInput")
    ns_d = nc.dram_tensor("negsel", [1, 1], fp32, kind="ExternalInput")
    st_o = nc.dram_tensor("stats", [8, 128, 8], fp32, kind="ExternalOutput")

    inv_t = float(1.0 / np.float32(TEMP))

    with TileContext(nc) as tc:
        with (
            tc.tile_pool(name="per", bufs=1) as per,
            tc.tile_pool(name="strip", bufs=2) as spp,
            tc.tile_pool(name="sm", bufs=2) as smp,
            tc.tile_pool(name="ps", bufs=8, space="PSUM") as psp,
        ):
            fa = per.tile([128, 2 * N], FEAT_DT, tag="fa")
            for k in range(2):
                for s in range(4):
                    nc.sync.dma_start(
                        out=fa[:, k * N + s * 2048:k * N + (s + 1) * 2048],
                        in_=fa_d[k][:, s * 2048:(s + 1) * 2048])
            rt = per.tile([128, 2 * NROW], FEAT_DT, tag="rt")
            for k in range(2):
                nc.sync.dma_start(out=rt[:, k * NROW:(k + 1) * NROW],
                                  in_=rt_d[k])
            nst = per.tile([1, 1], fp32, tag="nst")
            nc.sync.dma_start(out=nst[:1, :], in_=ns_d[:])
            nsb = per.tile([128, 1], fp32, tag="nsb")
            nc.gpsimd.partition_broadcast(nsb[:], nst[:1, :])
            npred = per.tile([128, 1], u8, tag="npred")
            nc.vector.tensor_scalar(npred[:], nsb[:], 0.5, None, op0=ALU.is_ge)

            for t in range(8):
                strip = spp.tile([128, N], fp32, tag="strip")
                sg16 = smp.tile([128, 16], fp32, tag="sg16")
                mx16 = smp.tile([128, 16], fp32, tag="mx16")
                for ng in range(4):
                    pss = [psp.tile([128, 512], fp32, tag="pss",
                                    name=f"pss{t}_{ng}_{j}")
                           for j in range(4)]
                    for k in range(2):
                        lh = rt[:, k * NROW + t * 128:k * NROW + (t + 1) * 128]
                        for ni in range(4):
                            n = ng * 4 + ni
                            nc.tensor.matmul(
                                pss[ni][:], lh,
                                fa[:, k * N + n * 512:k * N + (n + 1) * 512],
                                start=(k == 0), stop=(k == 1))
                    for ni in range(4):
                        n = ng * 4 + ni
                        nc.vector.tensor_scalar(
                            strip[:, n * 512:(n + 1) * 512], pss[ni][:],
                            inv_t, None, op0=ALU.mult, op1=ALU.add,
                            accum_out=sg16[:, n:n + 1])
                        nc.vector.tensor_reduce(
                            mx16[:, n:n + 1], strip[:, n * 512:(n + 1) * 512],
                            axis=AX.X, op=ALU.max)
                st8 = smp.tile([128, 8], fp32, tag="st8")
                rowmax = st8[:, 6:7]
                nc.vector.tensor_reduce(rowmax, mx16[:], axis=AX.X, op=ALU.max)
                maxneg = smp.tile([128, 1], fp32, tag="maxneg")
                nc.vector.tensor_scalar_mul(maxneg[:], rowmax, -1.0)
                # e = exp(l - max), per half, with exp-sums
                for h in range(2):
                    nc.scalar.activation(strip[:, h * HALF:(h + 1) * HALF],
                                         strip[:, h * HALF:(h + 1) * HALF],
                                         AF.Exp, bias=maxneg[:, :1], scale=1.0,
                                         accum_out=st8[:, h:h + 1])
                nsum = smp.tile([128, 1], fp32, tag="nsum")
                nc.vector.select(nsum[:], npred[:], st8[:, 1:2], st8[:, 0:1])
                # logterm = ln(e + negsum), per half, with sums
                for h in range(2):
                    nc.scalar.activation(strip[:, h * HALF:(h + 1) * HALF],
                                         strip[:, h * HALF:(h + 1) * HALF],
                                         AF.Ln, bias=nsum[:, :1], scale=1.0,
                                         accum_out=st8[:, 2 + h:3 + h])
                nc.vector.tensor_reduce(
                    st8[:, 4:6], sg16[:].rearrange("p (a b) -> p a b", a=2),
                    axis=AX.X, op=ALU.add)
                nc.vector.tensor_copy(st8[:, 7:8], nsum[:])
                nc.sync.dma_start(out=st_o[t], in_=st8[:])
    nc.compile()
    return nc


# --------------------------------------------------------------------------
# host orchestration
# --------------------------------------------------------------------------
def _run(nc, in_maps, tag):
    from concourse.bass_utils import run_bass_kernel_spmd
    trace = os.environ.get("KERNEL_TRACE", "0") == "1"
    t0 = time.monotonic()
    res = run_bass_kernel_spmd(nc, in_maps, core_ids=list(range(8)),
                               trace=trace)
    wall = time.monotonic() - t0
    last_exec_ns[tag] = {"exec_time_ns": res.exec_time_ns, "wall_s": wall}
    if trace and res.profile_json is not None:
        last_exec_ns[tag]["profile"] = True
    return res.results


def kernel(seg, embed, target, cam, class_weights):
    seg = np.ascontiguousarray(np.asarray(seg, dtype=np.float32))
    embed = np.ascontiguousarray(np.asarray(embed, dtype=np.float32))
    target = np.asarray(target)
    cam = np.ascontiguousarray(np.asarray(cam, dtype=np.float32))
    cw = np.ascontiguousarray(np.asarray(class_weights, dtype=np.float32))

    if "l1" not in _cache:
        _cache["l1"] = _build_l1()
    if "l2" not in _cache:
        _cache["l2"] = _build_l2()
    if "l3" not in _cache:
        _cache["l3"] = _build_l3()

    tgt_i32 = np.ascontiguousarray(target[:, 0].astype(np.int32))

    # ---- L1 ----
    in1 = [{"cam": np.ascontiguousarray(cam[b, 0]),
            "seg": np.ascontiguousarray(seg[b]),
            "tgt": tgt_i32[b], "cw": cw} for b in range(B)]
    r1 = _run(_cache["l1"], in1, "l1")

    ce_num = np.float32(0.0)
    ce_den = np.float32(0.0)
    idx_all = np.zeros((B, 8, 128), dtype=np.int32)
    for b in range(B):
        o = r1[b]
        ce_num += np.float32(o["ce"][:, 0].sum(dtype=np.float32))
        ce_den += np.float32(o["ce"][:, 1].sum(dtype=np.float32))
        fg_idx = np.flatnonzero(o["fgm"].reshape(-1))
        bg_idx = np.flatnonzero(o["bgm"].reshape(-1))
        if len(fg_idx) != NUM_SAMPLES or len(bg_idx) != NUM_SAMPLES:
            # numerically-tied threshold (measure-zero); host fallback
            cf = cam[b, 0].reshape(-1)
            fg_idx = np.argpartition(-cf, NUM_SAMPLES - 1)[:NUM_SAMPLES]
            bg_idx = np.argpartition(cf, NUM_SAMPLES - 1)[:NUM_SAMPLES]
        idx_all[b, :4] = np.sort(fg_idx).reshape(4, 128)
        idx_all[b, 4:] = np.sort(bg_idx).reshape(4, 128)
    ce = np.float32(ce_num / ce_den)

    # ---- L2 ----
    in2 = [{"embed": np.ascontiguousarray(embed[b].reshape(D, NPIX)),
            "idx": idx_all[b]} for b in range(B)]
    r2 = _run(_cache["l2"], in2, "l2")

    all_featsT = np.zeros((2, 128, N), dtype=FEAT_NP)
    normsq = np.zeros(N, dtype=np.float32)
    for b in range(B):
        ft = r2[b]["ft"]          # [2,128,1024]
        nr = r2[b]["nrm"]         # [128,8] col g = gather tile g
        fgc = slice(b * 512, (b + 1) * 512)
        bgc = slice(HALF + b * 512, HALF + (b + 1) * 512)
        all_featsT[:, :, fgc] = ft[:, :, 0:512]
        all_featsT[:, :, bgc] = ft[:, :, 512:1024]
        nrv = nr.T.reshape(-1)    # [1024] selected-row order
        normsq[fgc] = nrv[0:512]
        normsq[bgc] = nrv[512:1024]

    # ---- L3 ----
    in3 = [{"featsT": all_featsT,
            "rowsT": np.ascontiguousarray(
                all_featsT[:, :, c * NROW:(c + 1) * NROW]),
            "negsel": np.array([[1.0 if c < 4 else 0.0]], dtype=np.float32)}
           for c in range(8)]
    r3 = _run(_cache["l3"], in3, "l3")

    # ---- host finalization (exact fp32, mirrors jax semantics) ----
    ns0 = np.zeros(N, np.float32); ns1 = np.zeros(N, np.float32)
    L0 = np.zeros(N, np.float32); L1 = np.zeros(N, np.float32)
    SG0 = np.zeros(N, np.float32); SG1 = np.zeros(N, np.float32)
    maxl = np.zeros(N, np.float32)
    for c in range(8):
        s = r3[c]["stats"].reshape(NROW, 8)  # [8,128,8] -> row-major
        blk = slice(c * NROW, (c + 1) * NROW)
        ns0[blk] = s[:, 0]; ns1[blk] = s[:, 1]
        L0[blk] = s[:, 2]; L1[blk] = s[:, 3]
        SG0[blk] = s[:, 4]; SG1[blk] = s[:, 5]
        maxl[blk] = s[:, 6]
    is_fg = np.arange(N) < HALF
    negsum = np.where(is_fg, ns1, ns0).astype(np.float32)
    Tpos = np.where(is_fg, SG0 - np.float32(HALF) * maxl - L0,
                    SG1 - np.float32(HALF) * maxl - L1).astype(np.float32)
    with np.errstate(all="ignore"):
        l2ii = (normsq * np.float32(1.0 / np.float32(TEMP)) - maxl
                ).astype(np.float32)
        d = (l2ii - np.log(np.exp(l2ii) + negsum)).astype(np.float32)
        rows = np.where(negsum == 0.0, np.float32(np.nan),
                        (Tpos - d) / np.float32(N // 2 - 1)).astype(np.float32)
        contrast = np.float32(-(np.float32(TEMP) / np.float32(BASE_TEMP))
                              * np.mean(rows, dtype=np.float32))
        total = np.float32(ce + np.float32(LOSS_WEIGHT) * contrast)
    return (np.float32(total), np.float32(ce), np.float32(contrast))
